# revision 16
# baseline (speedup 1.0000x reference)
"""Bilinear distance kernel for Trainium2 (8 NeuronCores, SPMD).

dists[b,n,m] = sum_{i,j} data[b,n,i] * W[0,i,j] * crit[b,m,j]
B=16, N=M=2048, LD=RD=128, fp32.

Sharding: data-parallel over B (2 batches per core). Per batch:
  dataT[i,n] , critT[j,m]  via PE transposes (contraction dim -> partitions)
  lwT[j,n]  = W.T @ dataT          (GEMM1, W stationary)
  out[n,m]  = lwT_tile.T @ critT   (GEMM2, fp32r full-rate)

Output writes (32 MiB/core) are the memory roofline. Engine roles keep the
store pipeline unblocked: DVE does only GEMM2 PSUM->SBUF copies (in store
order), ACT does prep casts + all load DMAs, stores rotate sync/gpsimd.
"""

import sys

if "/opt/trn_rl_repo" not in sys.path:
    sys.path.insert(0, "/opt/trn_rl_repo")

import numpy as np

B, N, M, D = 16, 2048, 2048, 128
NCORES = 8
BPC = B // NCORES  # batches per core

_cache = {}


def _build():
    if "nc" in _cache:
        return _cache["nc"]

    import concourse.bacc as bacc
    import concourse.mybir as mybir
    from concourse import tile

    f32 = mybir.dt.float32
    f32r = mybir.dt.float32r

    nc = bacc.Bacc()
    data_d = nc.dram_tensor("data", [BPC, N, D], f32, kind="ExternalInput")
    crit_d = nc.dram_tensor("crit", [BPC, M, D], f32, kind="ExternalInput")
    w_d = nc.dram_tensor("w", [D, D], f32, kind="ExternalInput")
    out_d = nc.dram_tensor("out", [BPC, N, M], f32, kind="ExternalOutput")
    ident_d = nc.inline_tensor(np.eye(D, dtype=np.float32), name="ident")

    LG = 8               # row-groups per load DMA (1 MiB loads)
    NL = N // (128 * LG)
    # store group sizes (n-tiles per store DMA): small groups at the ends
    # (fast fill / short drain), 2-tile (2 MiB) groups in the steady state.
    GROUPS = [1, 1, 2, 2, 2, 2, 2, 2, 1, 1]
    assert sum(GROUPS) == N // 128

    cp = {"st": 0}

    with tile.TileContext(nc) as tc:
        store_rings = [nc.sync, nc.gpsimd]
        with (
            tc.tile_pool(name="const", bufs=1) as cpool,
            tc.tile_pool(name="loads", bufs=4) as lpool,
            tc.tile_pool(name="big", bufs=2) as bigpool,
            tc.tile_pool(name="outs", bufs=3) as opool,
            tc.tile_pool(name="pst", bufs=3, space="PSUM") as pst,
            tc.tile_pool(name="psg", bufs=1, space="PSUM") as psg,
            tc.tile_pool(name="ps2", bufs=2, space="PSUM") as ps2,
        ):
            w_raw = cpool.tile([D, D], f32)
            nc.gpsimd.dma_start(w_raw[:], w_d[:])
            w_sb = cpool.tile([D, D], f32r)
            nc.scalar.copy(w_sb[:], w_raw[:])
            ident = cpool.tile([D, D], f32)
            nc.gpsimd.dma_start(ident[:], ident_d[:])

            bigs = {}

            def alloc_big(b):
                bigs[b] = {
                    "dataT": bigpool.tile([D, N], f32r, tag="dataT", name=f"dataT{b}"),
                    "critT": bigpool.tile([D, M], f32r, tag="critT", name=f"critT{b}"),
                    "lwT": bigpool.tile([D, N], f32r, tag="lwT", name=f"lwT{b}"),
                }

            lds = {}

            def load(b):
                """Issue batch b's load DMAs (crit then data) on the ACT ring."""
                alloc_big(b)
                for src_d, key in ((crit_d, "critT"), (data_d, "dataT")):
                    for l in range(NL):
                        ld = lpool.tile(
                            [128, LG, D], f32, tag=key + "_ld", name=f"{key}_ld{b}{l}"
                        )
                        lds[(b, key, l)] = ld
                        nc.gpsimd.dma_start(
                            ld[:],
                            src_d[
                                b, l * LG * 128 : (l + 1) * LG * 128, :
                            ].rearrange("(g p) d -> p g d", p=128),
                        )

            def prep(b):
                """Transposes and GEMM1 for batch b (loads already issued).
                crit first (GEMM2 needs all of critT). Casts on ACT."""
                for src_d, key in ((crit_d, "critT"), (data_d, "dataT")):
                    dstT = bigs[b][key]
                    for l in range(NL):
                        ld = lds[(b, key, l)]
                        for g in range(LG):
                            ps = pst.tile([128, 128], f32, tag="pst", name="pst")
                            nc.tensor.transpose(ps[:], ld[:, g, :], ident[:])
                            t = l * LG + g
                            nc.scalar.copy(dstT[:, t * 128 : (t + 1) * 128], ps[:])
                    if key == "dataT":
                        for c in range(N // 512):
                            ps = psg.tile([128, 512], f32, tag="psg", name="psg")
                            nc.tensor.matmul(
                                ps[:],
                                w_sb[:],
                                dstT[:, c * 512 : (c + 1) * 512],
                                start=True,
                                stop=True,
                            )
                            nc.scalar.copy(
                                bigs[b]["lwT"][:, c * 512 : (c + 1) * 512], ps[:]
                            )

            def gemm2_groups(b, lo, hi):
                """Emit GEMM2 store groups [lo, hi) for batch b."""
                critT, lwT = bigs[b]["critT"], bigs[b]["lwT"]
                nt0 = sum(GROUPS[:lo])
                for gi in range(lo, hi):
                    sg = GROUPS[gi]
                    ot = opool.tile([128, 2, M], f32, tag="ot", name="ot")
                    for ntl in range(sg):
                        nt = nt0 + ntl
                        lhs = lwT[:, nt * 128 : (nt + 1) * 128]
                        for h in range(2):
                            p2 = ps2.tile([128, 1024], f32, tag="ps2", name="ps2")
                            for q in range(2):
                                mc = h * 1024 + q * 512
                                nc.tensor.matmul(
                                    p2[:, q * 512 : (q + 1) * 512],
                                    lhs,
                                    critT[:, mc : mc + 512],
                                    start=True,
                                    stop=True,
                                )
                            # late in the kernel ACT is done with prep casts;
                            # let it take every 3rd copy to unload DVE.
                            if cp["st"] >= 12 and (ntl * 2 + h) % 3 == 2:
                                nc.scalar.copy(
                                    ot[:, ntl, h * 1024 : (h + 1) * 1024], p2[:]
                                )
                            else:
                                nc.vector.tensor_copy(
                                    ot[:, ntl, h * 1024 : (h + 1) * 1024], p2[:]
                                )
                    ring = store_rings[cp["st"] % 2]
                    cp["st"] += 1
                    ring.dma_start(
                        out_d[b, nt0 * 128 : (nt0 + sg) * 128, :].rearrange(
                            "(g p) m -> p g m", p=128
                        ),
                        ot[:, :sg, :],
                    )
                    nt0 += sg

            NG = len(GROUPS)
            for b in range(BPC):
                load(b)
            prep(0)
            gemm2_groups(0, 0, 4)
            for b in range(BPC):
                if b + 1 < BPC:
                    prep(b + 1)
                    gemm2_groups(b, 4, NG)
                    gemm2_groups(b + 1, 0, 4)
                else:
                    gemm2_groups(b, 4, NG)

    nc.finalize()
    _cache["nc"] = nc
    return nc


def kernel(data: np.ndarray, crit: np.ndarray, W: np.ndarray) -> np.ndarray:
    from concourse.bass_utils import run_bass_kernel_spmd

    nc = _build()
    data = np.ascontiguousarray(data, dtype=np.float32)
    crit = np.ascontiguousarray(crit, dtype=np.float32)
    w = np.ascontiguousarray(W.reshape(D, D), dtype=np.float32)
    in_maps = [
        {
            "data": data[c * BPC : (c + 1) * BPC],
            "crit": crit[c * BPC : (c + 1) * BPC],
            "w": w,
        }
        for c in range(NCORES)
    ]
    res = run_bass_kernel_spmd(nc, in_maps, core_ids=list(range(NCORES)))
    return np.concatenate([r["out"] for r in res.results], axis=0)


# revision 17
# speedup vs baseline: 1.0230x; 1.0230x over previous
"""Bilinear distance kernel for Trainium2 (8 NeuronCores, SPMD).

dists[b,n,m] = sum_{i,j} data[b,n,i] * W[0,i,j] * crit[b,m,j]
B=16, N=M=2048, LD=RD=128, fp32.

Sharding: data-parallel over B (2 batches per core). Per batch:
  dataT[i,n] , critT[j,m]  via PE transposes (contraction dim -> partitions)
  lwT[j,n]  = W.T @ dataT          (GEMM1, W stationary)
  out[n,m]  = lwT_tile.T @ critT   (GEMM2, fp32r full-rate)

Output writes (32 MiB/core) are the memory roofline. Four 128x128 PE
transposes share one PSUM bank so each PSUM->SBUF cast moves [128,512].
"""

import sys

if "/opt/trn_rl_repo" not in sys.path:
    sys.path.insert(0, "/opt/trn_rl_repo")

import numpy as np

B, N, M, D = 16, 2048, 2048, 128
NCORES = 8
BPC = B // NCORES  # batches per core

_cache = {}


def _build():
    if "nc" in _cache:
        return _cache["nc"]

    import concourse.bacc as bacc
    import concourse.mybir as mybir
    from concourse import tile

    f32 = mybir.dt.float32
    f32r = mybir.dt.float32r

    nc = bacc.Bacc()
    data_d = nc.dram_tensor("data", [BPC, N, D], f32, kind="ExternalInput")
    crit_d = nc.dram_tensor("crit", [BPC, M, D], f32, kind="ExternalInput")
    w_d = nc.dram_tensor("w", [D, D], f32, kind="ExternalInput")
    out_d = nc.dram_tensor("out", [BPC, N, M], f32, kind="ExternalOutput")
    ident_d = nc.inline_tensor(np.eye(D, dtype=np.float32), name="ident")

    LG = 8               # row-groups per load DMA (1 MiB loads)
    NL = N // (128 * LG)
    # store group sizes (n-tiles per store DMA): small groups at the ends
    # (fast fill / short drain), 2-tile (2 MiB) groups in the steady state.
    GROUPS = [1, 1, 2, 2, 2, 2, 2, 2, 1, 1]
    assert sum(GROUPS) == N // 128

    cp = {"k": 0, "st": 0}

    with tile.TileContext(nc) as tc:
        with (
            tc.tile_pool(name="const", bufs=1) as cpool,
            tc.tile_pool(name="loads", bufs=2) as lpool,
            tc.tile_pool(name="big", bufs=2) as bigpool,
            tc.tile_pool(name="outs", bufs=3) as opool,
            tc.tile_pool(name="pst", bufs=3, space="PSUM") as pst,
            tc.tile_pool(name="psg", bufs=1, space="PSUM") as psg,
            tc.tile_pool(name="ps2", bufs=2, space="PSUM") as ps2,
        ):
            w_raw = cpool.tile([D, D], f32)
            nc.gpsimd.dma_start(w_raw[:], w_d[:])
            w_sb = cpool.tile([D, D], f32r)
            nc.vector.tensor_copy(w_sb[:], w_raw[:])
            ident = cpool.tile([D, D], f32)
            nc.gpsimd.dma_start(ident[:], ident_d[:])

            for b in range(BPC):
                dataT = bigpool.tile([D, N], f32r, tag="dataT", name=f"dataT{b}")
                critT = bigpool.tile([D, M], f32r, tag="critT", name=f"critT{b}")
                lwT = bigpool.tile([D, N], f32r, tag="lwT", name=f"lwT{b}")

                # ---- load + transpose crit then data; 4 transposes share a
                # PSUM bank -> one [128,512] cast each.
                for src_d, dstT, ldeng in (
                    (crit_d, critT, nc.sync),
                    (data_d, dataT, nc.scalar),
                ):
                    for l in range(NL):
                        ld = lpool.tile(
                            [128, LG, D], f32, tag=f"ld{src_d.name}", name="ld"
                        )
                        ldeng.dma_start(
                            ld[:],
                            src_d[
                                b, l * LG * 128 : (l + 1) * LG * 128, :
                            ].rearrange("(g p) d -> p g d", p=128),
                        )
                        for q4 in range(LG // 4):
                            ps = pst.tile([128, 512], f32, tag="pst", name="pst")
                            for k in range(4):
                                nc.tensor.transpose(
                                    ps[:, k * 128 : (k + 1) * 128],
                                    ld[:, q4 * 4 + k, :],
                                    ident[:],
                                )
                            c0 = (l * LG + q4 * 4) * 128
                            nc.vector.tensor_copy(dstT[:, c0 : c0 + 512], ps[:])

                # ---- GEMM1: lwT[j, n] = W.T @ dataT
                for c in range(N // 512):
                    ps = psg.tile([128, 512], f32, tag="psg", name="psg")
                    nc.tensor.matmul(
                        ps[:],
                        w_sb[:],
                        dataT[:, c * 512 : (c + 1) * 512],
                        start=True,
                        stop=True,
                    )
                    nc.vector.tensor_copy(lwT[:, c * 512 : (c + 1) * 512], ps[:])

                # ---- GEMM2: out[n0:n0+128, :] = lwT_tile.T @ critT
                nt0 = 0
                for sg in GROUPS:
                    ot = opool.tile([128, 2, M], f32, tag="ot", name="ot")
                    for ntl in range(sg):
                        nt = nt0 + ntl
                        lhs = lwT[:, nt * 128 : (nt + 1) * 128]
                        for h in range(2):
                            p2 = ps2.tile([128, 1024], f32, tag="ps2", name="ps2")
                            for q in range(2):
                                mc = h * 1024 + q * 512
                                nc.tensor.matmul(
                                    p2[:, q * 512 : (q + 1) * 512],
                                    lhs,
                                    critT[:, mc : mc + 512],
                                    start=True,
                                    stop=True,
                                )
                            eng = nc.vector if cp["k"] % 3 != 1 else nc.scalar
                            cp["k"] += 1
                            if eng is nc.vector:
                                eng.tensor_copy(
                                    ot[:, ntl, h * 1024 : (h + 1) * 1024], p2[:]
                                )
                            else:
                                eng.copy(ot[:, ntl, h * 1024 : (h + 1) * 1024], p2[:])
                    st_eng = nc.sync if cp["st"] % 2 == 0 else nc.scalar
                    cp["st"] += 1
                    st_eng.dma_start(
                        out_d[b, nt0 * 128 : (nt0 + sg) * 128, :].rearrange(
                            "(g p) m -> p g m", p=128
                        ),
                        ot[:, :sg, :],
                    )
                    nt0 += sg

    nc.finalize()
    _cache["nc"] = nc
    return nc


def kernel(data: np.ndarray, crit: np.ndarray, W: np.ndarray) -> np.ndarray:
    from concourse.bass_utils import run_bass_kernel_spmd

    nc = _build()
    data = np.ascontiguousarray(data, dtype=np.float32)
    crit = np.ascontiguousarray(crit, dtype=np.float32)
    w = np.ascontiguousarray(W.reshape(D, D), dtype=np.float32)
    in_maps = [
        {
            "data": data[c * BPC : (c + 1) * BPC],
            "crit": crit[c * BPC : (c + 1) * BPC],
            "w": w,
        }
        for c in range(NCORES)
    ]
    res = run_bass_kernel_spmd(nc, in_maps, core_ids=list(range(NCORES)))
    return np.concatenate([r["out"] for r in res.results], axis=0)
